# revision 16
# baseline (speedup 1.0000x reference)
"""Trainium2 Bass kernel: LocalEmbeddingLayer (KNN -> gather -> 2-layer GELU MLP -> mean).

Full-input contract: kernel(**inputs) takes the unsharded inputs and returns the
full [B, N, P] output. Internally shards batch B=32 across 8 NeuronCores (pure
data parallel, 4 batch elements per core), runs one SPMD Bass program on all
cores, and concatenates the per-core outputs.

v4 design (v1 f32r baseline 1840 us, v2 split-gather 609 us):

* Gather: ap_gather's cost is ~num_idxs per 16-partition Q7 core group, so
  the 2048 gathered (query,k) columns of a block are split 4 ways across
  the partition groups (chunk a = queries 32a..32a+32 on partitions
  32a..32a+32, 64 features f16-packed 2-per-index, d=2).  num_idxs=512 ->
  ~9 us/block vs ~35 us unsplit (which serialized every engine in v1).
* Tensor is then the pacing engine at ~515 ns per 512-column matmul
  (fp16/bf16 stream ~0.85 ns/col + issue overhead; LDWEIGHTS is hidden).
  v2 spent 35 matmuls/block because the packed-pair layout forced 32-deep
  contractions re-streamed per h-half (24 L1 matmuls).  v4 un-scrambles
  the gather chunks ONCE with 8 selection matmuls (identity weights, PSUM
  -> act-copy into featM[0:64]), DMAs the 16x-broadcast center features
  straight from DRAM into featM[64:128] (host-precomputed, no engine
  time), and runs layer 1 as a single 128-deep contraction: 3 dist + 8
  deint + 8 L1 + 8 L2 = 27 matmuls/block.
* h1 = W1a^T nbr + (W1b-W1a)^T ctr + b1 with the center subtraction folded
  into host-prepared weights; W1a rows permuted to the deint order
  (featM row P holds feature 2*(P%32) + P//32).
* 6-deep software pipeline over the 32 row-blocks:
    A(s)   dist matmul + top-16 + index transposes   [PE + DVE]
    B(s-1) ap_gather                                  [GpSimd]
    C(s-2) deint matmuls + copies + center DMA        [PE + Act + DMA]
    D(s-3) 2-layer f16 MLP                            [PE + Act]
    E(s-4) mean over k + scale + store                [DVE + Act + DMA]
"""

import numpy as np

B, N, DPOS, F, P, K = 32, 1024, 3, 64, 128, 16
NCORES = 8
BL = B // NCORES          # batches per core
NBLK = N // 128           # row blocks per batch
NEG = -1.0e30


def build_program(gelu=True, n_b=BL, n_blk=NBLK):
    import concourse.bacc as bacc
    import concourse.mybir as mybir
    from concourse.tile import TileContext

    f32 = mybir.dt.float32
    f16 = mybir.dt.float16
    bf16 = mybir.dt.bfloat16
    u16 = mybir.dt.uint16
    i16 = mybir.dt.int16
    AF = mybir.ActivationFunctionType
    act_fn = AF.Gelu if gelu else AF.Identity

    nc = bacc.Bacc("TRN2", target_bir_lowering=False)

    featP_d = nc.dram_tensor("featP", [n_b, 128, 2 * N], f16, kind="ExternalInput")
    featB_d = nc.dram_tensor("featB", [n_b, 64, K * N], f16, kind="ExternalInput")
    ab_d = nc.dram_tensor("ab", [n_b, 2, 16, N], bf16, kind="ExternalInput")
    sel_d = nc.dram_tensor("sel", [128, 128], f16, kind="ExternalInput")
    w1_d = nc.dram_tensor("w1", [128, 256], f16, kind="ExternalInput")
    w2_d = nc.dram_tensor("w2", [128, 256], f16, kind="ExternalInput")
    b1_d = nc.dram_tensor("b1", [128, 2], f32, kind="ExternalInput")
    b2_d = nc.dram_tensor("b2", [128, 1], f32, kind="ExternalInput")
    cbf_d = nc.dram_tensor("cbf", [128, 256], bf16, kind="ExternalInput")
    out_d = nc.dram_tensor("out", [n_b, n_blk, 128, 128], f32, kind="ExternalOutput")

    with TileContext(nc) as tc:
        with (
            tc.tile_pool(name="const", bufs=1) as cpool,
            tc.tile_pool(name="feat", bufs=2) as fpool,
            tc.tile_pool(name="nbuf", bufs=2) as npool,
            tc.tile_pool(name="mbuf", bufs=3) as mpool,
            tc.tile_pool(name="gbuf", bufs=2) as gpool,
            tc.tile_pool(name="hbuf", bufs=2) as hpool,
            tc.tile_pool(name="small", bufs=3) as spool,
            tc.tile_pool(name="ps_tk", bufs=2, space="PSUM") as ptk,
            tc.tile_pool(name="ps_h1", bufs=2, space="PSUM") as ph1,
            tc.tile_pool(name="ps_b", bufs=2, space="PSUM") as pb,
        ):
            sel_sb = cpool.tile([128, 128], f16)
            nc.sync.dma_start(sel_sb[:], sel_d[:])
            w1_sb = cpool.tile([128, 256], f16)
            nc.sync.dma_start(w1_sb[:], w1_d[:])
            w2_sb = cpool.tile([128, 256], f16)
            nc.sync.dma_start(w2_sb[:], w2_d[:])
            b1_sb = cpool.tile([128, 2], f32)
            nc.sync.dma_start(b1_sb[:], b1_d[:])
            b2_sb = cpool.tile([128, 1], f32)
            nc.sync.dma_start(b2_sb[:], b2_d[:])
            cbf_sb = cpool.tile([128, 256], bf16)   # cols 0:128 I, 128:256 -1e30*I
            nc.sync.dma_start(cbf_sb[:], cbf_d[:])

            feat_tiles = {}

            itile_pairs = {}

            def stageA(b, blk, s):
                """Distance scores, top-16, gather-index tile for one block."""
                if blk == 0:
                    fp = fpool.tile([128, 2 * N], f16, tag="featP")
                    nc.sync.dma_start(fp[:], featP_d[b])
                    ab = fpool.tile([16, 2 * N], bf16, tag="ab")
                    nc.sync.dma_start(
                        ab[:].rearrange("d (x n) -> d x n", x=2),
                        ab_d[b].rearrange("x d n -> d x n"),
                    )
                    feat_tiles[b] = (fp, ab)
                _, ab_sb = feat_tiles[b]

                # Tk[i,j] = 2*p_i.p_j - |p_j|^2 via bf16 hi/lo split; self
                # masked by accumulating -1e30*I onto the diagonal block.
                tk_ps = ptk.tile([128, N], f32, tag="tkps")
                lhsA = ab_sb[:, blk * 128:(blk + 1) * 128]
                for h in range(2):
                    nc.tensor.matmul(
                        tk_ps[:, h * 512:(h + 1) * 512],
                        lhsA,
                        ab_sb[:, N + h * 512:N + (h + 1) * 512],
                        start=True, stop=True,
                    )
                nc.tensor.matmul(
                    tk_ps[:, blk * 128:(blk + 1) * 128],
                    cbf_sb[:, 0:128],
                    cbf_sb[:, 128:256],
                    start=False, stop=True,
                    skip_group_check=True,
                )

                # top-16 per row, operating directly on PSUM
                vals = spool.tile([128, 16], f32, tag="vals")
                idxp = spool.tile([128, 32], u16, tag="idxp")
                nc.vector.max(vals[:, 0:8], tk_ps[:])
                nc.vector.max_index(idxp[:, 0:8], vals[:, 0:8], tk_ps[:])
                nc.vector.match_replace(tk_ps[:], vals[:, 0:8], tk_ps[:], NEG)
                nc.vector.max(vals[:, 8:16], tk_ps[:])
                nc.vector.max_index(idxp[:, 8:16], vals[:, 8:16], tk_ps[:])
                # each 16-partition core group needs its own copy of the
                # wrapped index list -> duplicate before the 32x32 transpose
                nc.vector.tensor_copy(idxp[:, 16:32], idxp[:, 0:16])

                # itile[32a+p, 32h+w]: p in 0:16 -> idx[q=32a+w, k=p] of the
                # pair's block h for core 2a, p in 16:32 the same indices
                # again for core 2a+1.  Two blocks share one index tile so
                # one ap_gather serves both (the ucode has ~7 us fixed
                # overhead per launch + ~13.5 ns/idx; pairing halves the
                # fixed cost per block).
                t = s // 2
                if s % 2 == 0:
                    itile2 = spool.tile([128, 64], u16, tag="itile")
                    itile_pairs[t] = itile2
                itile2 = itile_pairs[t]
                half = s % 2
                for a in range(4):
                    nc.vector.transpose(
                        itile2[32 * a:32 * (a + 1), 32 * half:32 * (half + 1)],
                        idxp[32 * a:32 * (a + 1), :],
                    )

            def stageB(b, t):
                """Gather neighbor features for block pair t: chunk a of each
                block's 2048 (query, k) columns lands on partitions
                32a:32a+32, features f16-packed 2-per-index."""
                fp, _ = feat_tiles[b]
                nb2 = npool.tile([128, 1024, 2], f16, tag="nb2")
                nc.gpsimd.ap_gather(
                    nb2[:],
                    fp[:].rearrange("p (n j) -> p n j", j=2),
                    itile_pairs.pop(t)[:].bitcast(i16),
                    channels=128, num_elems=N, d=2, num_idxs=1024,
                )
                return nb2

            def stageC(b, blk, nb2, half):
                """Un-scramble the gather chunks into featM: selection
                matmuls move feature 2cc+j from chunk partition 32a+cc to
                featM row cc+32j for the chunk's 512 columns (one pass for
                both h-halves); featM[64:128] gets the 16x-broadcast center
                features straight from DRAM."""
                featM = mpool.tile([128, 2048], f16, tag="featM")
                off = 512 * half
                for a in range(4):
                    # shares the ps_b ring with the L2 tiles (same tag) so the
                    # budget stays within 8 PSUM banks with tk double-buffered
                    fm = pb.tile([128, 512], f32, tag="pb")
                    for j in range(2):
                        nc.tensor.matmul(
                            fm[0:64, :],
                            sel_sb[32 * a:32 * (a + 1), 64 * j:64 * (j + 1)],
                            nb2[32 * a:32 * (a + 1), off:off + 512, j:j + 1],
                            start=(j == 0), stop=(j == 1),
                            tile_position=(32 * a, 0),
                        )
                    if a % 2 == 0:
                        nc.scalar.activation(
                            featM[0:64, 512 * a:512 * (a + 1)], fm[0:64, :],
                            AF.Copy,
                        )
                    else:
                        # split the PSUM->SBUF drains between Act and DVE so
                        # neither engine head-of-line-blocks the psum ring
                        nc.vector.tensor_copy(
                            featM[0:64, 512 * a:512 * (a + 1)], fm[0:64, :]
                        )
                nc.sync.dma_start(
                    featM[64:128, :],
                    featB_d[b, :, blk * 2048:(blk + 1) * 2048],
                )
                return featM

            def stageD(b, blk, featM):
                """f16 MLP, single 128-deep contraction per (h-half, q-chunk):
                h1 = W1a^T nbr + (W1b-W1a)^T ctr + b1; out = gelu(W2^T gelu(h1) + b2)."""
                g2 = gpool.tile([128, 2048], f16, tag="g2")
                hs0 = hpool.tile([128, 2048], f16, tag="hs0")
                hs1 = hpool.tile([128, 2048], f16, tag="hs1")
                hs_pair = [hs0, hs1]
                for c in range(4):
                    cols = slice(512 * c, 512 * (c + 1))
                    for hh in range(2):
                        hp = ph1.tile([128, 512], f32, tag="h1ps")
                        nc.tensor.matmul(
                            hp[:],
                            w1_sb[:, hh * 128:(hh + 1) * 128],
                            featM[:, cols],
                            start=True, stop=True,
                        )
                        nc.scalar.activation(
                            hs_pair[hh][:, cols], hp[:], act_fn,
                            bias=b1_sb[:, hh:hh + 1],
                        )
                    p2 = pb.tile([128, 512], f32, tag="pb")
                    for hh in range(2):
                        nc.tensor.matmul(
                            p2[:],
                            w2_sb[:, hh * 128:(hh + 1) * 128],
                            hs_pair[hh][:, cols],
                            start=(hh == 0), stop=(hh == 1),
                        )
                    nc.scalar.activation(
                        g2[:, cols], p2[:], act_fn, bias=b2_sb[:, 0:1],
                    )
                return g2

            def stageE(b, blk, g2):
                """Mean over the k=16 neighbors, scale, store [P, r]."""
                red = spool.tile([128, 128], f32, tag="red")
                nc.vector.tensor_reduce(
                    red[:], g2[:].rearrange("p (r k) -> p r k", k=K),
                    axis=mybir.AxisListType.X, op=mybir.AluOpType.add,
                )
                outT = spool.tile([128, 128], f32, tag="outT")
                nc.vector.tensor_scalar_mul(outT[:], red[:], 1.0 / K)
                nc.sync.dma_start(out_d[b, blk], outT[:])

            S = n_b * n_blk
            B_out, C_out, D_out = {}, {}, {}
            for s in range(S + 4):
                if s < S:
                    stageA(*divmod(s, n_blk), s)
                    if s % 2 == 1:
                        t = s // 2
                        B_out[t] = stageB((2 * t) // n_blk, t)
                if 2 <= s <= S + 1:
                    sc = s - 2
                    C_out[sc] = stageC(*divmod(sc, n_blk), B_out[sc // 2], sc % 2)
                    if sc % 2 == 1:
                        B_out.pop(sc // 2)
                if 3 <= s <= S + 2:
                    D_out[s - 3] = stageD(*divmod(s - 3, n_blk), C_out.pop(s - 3))
                if s >= 4:
                    stageE(*divmod(s - 4, n_blk), D_out.pop(s - 4))

    nc.compile()
    return nc


def prep_core_inputs(points, features, W1, b1, W2, b2, core):
    """Host-side packing of one core's inputs (batches core*BL .. core*BL+BL)."""
    import ml_dtypes
    bf = ml_dtypes.bfloat16
    sl = slice(core * BL, (core + 1) * BL)
    pts = points[sl]           # [BL, N, 3]
    fts = features[sl]         # [BL, N, F]

    fT = np.ascontiguousarray(fts.transpose(0, 2, 1))        # [BL, 64, N]
    featC = fT.astype(np.float16)
    # featP[b, 32a+cc, 2n+j] = feat[b, 2cc+j, n], replicated over a=0..3
    packed = featC.reshape(BL, 32, 2, N).transpose(0, 1, 3, 2)
    featP = np.tile(packed.reshape(BL, 32, 2 * N), (1, 4, 1))
    # featB[b, f, 16n+k] = feat[b, f, n]: 16x-broadcast center features
    featB = np.repeat(featC, K, axis=2)

    r = (pts.astype(np.float64) ** 2).sum(-1).astype(np.float32)  # [BL, N]
    p_hi = pts.astype(bf).astype(np.float32)
    p_lo = (pts - p_hi).astype(bf).astype(np.float32)
    r_hi = r.astype(bf).astype(np.float32)
    r_lo = (r - r_hi).astype(bf).astype(np.float32)

    ab = np.zeros((BL, 2, 16, N), np.float32)
    # lhs rows (A) pair with rhs rows (B); Tk = 2 p_i . p_j - r_j
    ab[:, 0, 0:3] = 2.0 * p_hi.transpose(0, 2, 1)
    ab[:, 0, 3:6] = 2.0 * p_lo.transpose(0, 2, 1)
    ab[:, 0, 6:9] = 2.0 * p_hi.transpose(0, 2, 1)
    ab[:, 0, 9] = -1.0
    ab[:, 0, 10] = -1.0
    ab[:, 1, 0:3] = p_hi.transpose(0, 2, 1)
    ab[:, 1, 3:6] = p_hi.transpose(0, 2, 1)
    ab[:, 1, 6:9] = p_lo.transpose(0, 2, 1)
    ab[:, 1, 9] = r_hi
    ab[:, 1, 10] = r_lo
    ab = ab.astype(bf)

    # selection weights: move chunk partition 32a+cc, pair-lane j to featM
    # row cc+32j.  sel[32a+cc, 64j + (cc+32j)] = 1.
    sel = np.zeros((128, 128), np.float16)
    cc = np.arange(32)
    for a in range(4):
        sel[32 * a + cc, cc] = 1.0            # j=0 -> rows cc
        sel[32 * a + cc, 64 + 32 + cc] = 1.0  # j=1 -> rows 32+cc
    # w1 rows 0:64: W1a with row P holding feature 2*(P%32)+P//32 (the deint
    # order); rows 64:128: W1b - W1a in natural order.
    w1p = np.empty((128, 256), np.float32)
    w1p[0:64] = W1[0:64].reshape(32, 2, 256).transpose(1, 0, 2).reshape(64, 256)
    w1p[64:128] = W1[64:128] - W1[0:64]
    w1p = w1p.astype(np.float16)
    w2p = np.empty((128, 256), np.float32)
    w2p[:, 0:128] = W2[0:128]
    w2p[:, 128:256] = W2[128:256]
    w2p = w2p.astype(np.float16)
    b1p = np.ascontiguousarray(b1.reshape(2, 128).T)
    b2p = np.ascontiguousarray(b2.reshape(128, 1))

    eye = np.eye(128, dtype=np.float32)
    cbf = np.concatenate([eye, NEG * eye], axis=1).astype(bf)

    return {
        "featP": np.ascontiguousarray(featP),
        "featB": np.ascontiguousarray(featB),
        "ab": np.ascontiguousarray(ab),
        "sel": sel,
        "w1": w1p, "w2": w2p, "b1": b1p, "b2": b2p,
        "cbf": np.ascontiguousarray(cbf),
    }


_CACHED = {}


def kernel(points, features, W1, b1, W2, b2):
    from concourse import bass_utils

    points = np.asarray(points, np.float32)
    features = np.asarray(features, np.float32)
    W1 = np.asarray(W1, np.float32)
    b1 = np.asarray(b1, np.float32)
    W2 = np.asarray(W2, np.float32)
    b2 = np.asarray(b2, np.float32)

    if "nc" not in _CACHED:
        _CACHED["nc"] = build_program(gelu=True)
    nc = _CACHED["nc"]

    in_maps = [
        prep_core_inputs(points, features, W1, b1, W2, b2, c)
        for c in range(NCORES)
    ]
    res = bass_utils.run_bass_kernel_spmd(
        nc, in_maps, core_ids=list(range(NCORES))
    )
    outs = []
    for c in range(NCORES):
        o = res.results[c]["out"]          # [BL, NBLK, 128, 128] = [b, blk, P, r]
        outs.append(o.transpose(0, 1, 3, 2).reshape(BL, N, P))
    return np.concatenate(outs, axis=0)


# revision 17
# speedup vs baseline: 1.0616x; 1.0616x over previous
"""Trainium2 Bass kernel: LocalEmbeddingLayer (KNN -> gather -> 2-layer GELU MLP -> mean).

Full-input contract: kernel(**inputs) takes the unsharded inputs and returns the
full [B, N, P] output. Internally shards batch B=32 across 8 NeuronCores (pure
data parallel, 4 batch elements per core), runs one SPMD Bass program on all
cores, and concatenates the per-core outputs.

v7 design (v1 baseline 1840 us, v2 609, v5 522):

* The ap_gather ucode is the pacemaker: measured ~7.3 us fixed launch cost
  + ~13.5 ns per index per 16-partition core group, serialized on the one
  GpSimd engine.  (d=2/f16 at 1024 idxs goes super-linear -- 48.8 us --
  so pairs are packed as ONE f32 word and gathered with d=1/f32, the exact
  configuration v1 measured at 35 us for 2048 idxs.)  One gather serves
  FOUR row-blocks (num_idxs=2048) -> 8.75 us/block amortized.
* The 2048 gathered (query,k) columns of each block are split 4 ways
  across the partition groups (chunk a = queries 32a..32a+32 of the block
  on partitions 32a..32a+32, 64 features packed 2-per-f32-index).
* Un-scramble per block: 8 selection matmuls (identity weights, 64-out
  tiles stream at ~2x the 128-out rate) -> PSUM -> Act/DVE copies into
  featM[0:64]; featM[64:128] gets the 16x-broadcast center features
  straight from DRAM (host-precomputed, no engine time).  Layer 1 is then
  a single 128-deep contraction: 3 dist + 8 deint + 8 L1 + 8 L2 = 27
  matmuls/block at ~0.85 ns/col + ~100 ns issue overhead (Tensor ~13 us).
* h1 = W1a^T nbr + (W1b-W1a)^T ctr + b1 with the center subtraction folded
  into host-prepared weights; W1a rows permuted to the deint order
  (featM row P holds feature 2*(P%32) + P//32).
* 8-deep software pipeline over the 32 row-blocks:
    A(s)    dist matmul + top-16 + index transposes  [PE + DVE]
    B(s//4) ap_gather for 4 blocks, after A(4T+3)    [GpSimd]
    C(s-5)  deint matmuls + copies + center DMA      [PE + Act/DVE + DMA]
    D(s-6)  2-layer f16 MLP                          [PE + Act]
    E(s-7)  mean over k + scale + store              [DVE + DMA]
"""

import numpy as np

B, N, DPOS, F, P, K = 32, 1024, 3, 64, 128, 16
NCORES = 8
BL = B // NCORES          # batches per core
NBLK = N // 128           # row blocks per batch
NEG = -1.0e30


def build_program(gelu=True, n_b=BL, n_blk=NBLK):
    import concourse.bacc as bacc
    import concourse.mybir as mybir
    from concourse.tile import TileContext

    f32 = mybir.dt.float32
    f16 = mybir.dt.float16
    bf16 = mybir.dt.bfloat16
    u16 = mybir.dt.uint16
    i16 = mybir.dt.int16
    AF = mybir.ActivationFunctionType
    act_fn = AF.Gelu if gelu else AF.Identity

    nc = bacc.Bacc("TRN2", target_bir_lowering=False)

    featP_d = nc.dram_tensor("featP", [n_b, 128, 2 * N], f16, kind="ExternalInput")
    featB_d = nc.dram_tensor("featB", [n_b, 64, K * N], f16, kind="ExternalInput")
    ab_d = nc.dram_tensor("ab", [n_b, 2, 16, N], bf16, kind="ExternalInput")
    sel_d = nc.dram_tensor("sel", [128, 128], f16, kind="ExternalInput")
    w1_d = nc.dram_tensor("w1", [128, 256], f16, kind="ExternalInput")
    w2_d = nc.dram_tensor("w2", [128, 256], f16, kind="ExternalInput")
    b1_d = nc.dram_tensor("b1", [128, 2], f32, kind="ExternalInput")
    b2_d = nc.dram_tensor("b2", [128, 1], f32, kind="ExternalInput")
    cbf_d = nc.dram_tensor("cbf", [128, 256], bf16, kind="ExternalInput")
    out_d = nc.dram_tensor("out", [n_b, n_blk, 128, 128], f32, kind="ExternalOutput")

    with TileContext(nc) as tc:
        with (
            tc.tile_pool(name="const", bufs=1) as cpool,
            tc.tile_pool(name="feat", bufs=2) as fpool,
            tc.tile_pool(name="nbuf", bufs=2) as npool,
            tc.tile_pool(name="mbuf", bufs=3) as mpool,
            tc.tile_pool(name="gbuf", bufs=2) as gpool,
            tc.tile_pool(name="hbuf", bufs=2) as hpool,
            tc.tile_pool(name="small", bufs=3) as spool,
            tc.tile_pool(name="ps_tk", bufs=2, space="PSUM") as ptk,
            tc.tile_pool(name="ps_h1", bufs=2, space="PSUM") as ph1,
            tc.tile_pool(name="ps_b", bufs=2, space="PSUM") as pb,
        ):
            sel_sb = cpool.tile([128, 128], f16)
            nc.sync.dma_start(sel_sb[:], sel_d[:])
            w1_sb = cpool.tile([128, 256], f16)
            nc.sync.dma_start(w1_sb[:], w1_d[:])
            w2_sb = cpool.tile([128, 256], f16)
            nc.sync.dma_start(w2_sb[:], w2_d[:])
            b1_sb = cpool.tile([128, 2], f32)
            nc.sync.dma_start(b1_sb[:], b1_d[:])
            b2_sb = cpool.tile([128, 1], f32)
            nc.sync.dma_start(b2_sb[:], b2_d[:])
            cbf_sb = cpool.tile([128, 256], bf16)   # cols 0:128 I, 128:256 -1e30*I
            nc.sync.dma_start(cbf_sb[:], cbf_d[:])

            feat_tiles = {}
            itile_groups = {}

            def stageA(b, blk, s):
                """Distance scores, top-16, gather-index tile for one block."""
                if blk == 0:
                    fp = fpool.tile([128, 2 * N], f16, tag="featP")
                    nc.sync.dma_start(fp[:], featP_d[b])
                    ab = fpool.tile([16, 2 * N], bf16, tag="ab")
                    nc.sync.dma_start(
                        ab[:].rearrange("d (x n) -> d x n", x=2),
                        ab_d[b].rearrange("x d n -> d x n"),
                    )
                    feat_tiles[b] = (fp, ab)
                _, ab_sb = feat_tiles[b]

                # Tk[i,j] = 2*p_i.p_j - |p_j|^2 via bf16 hi/lo split; self
                # masked by accumulating -1e30*I onto the diagonal block.
                tk_ps = ptk.tile([128, N], f32, tag="tkps")
                lhsA = ab_sb[:, blk * 128:(blk + 1) * 128]
                for h in range(2):
                    nc.tensor.matmul(
                        tk_ps[:, h * 512:(h + 1) * 512],
                        lhsA,
                        ab_sb[:, N + h * 512:N + (h + 1) * 512],
                        start=True, stop=True,
                    )
                nc.tensor.matmul(
                    tk_ps[:, blk * 128:(blk + 1) * 128],
                    cbf_sb[:, 0:128],
                    cbf_sb[:, 128:256],
                    start=False, stop=True,
                    skip_group_check=True,
                )

                # top-16 per row, operating directly on PSUM
                vals = spool.tile([128, 16], f32, tag="vals")
                idxp = spool.tile([128, 32], u16, tag="idxp")
                nc.vector.max(vals[:, 0:8], tk_ps[:])
                nc.vector.max_index(idxp[:, 0:8], vals[:, 0:8], tk_ps[:])
                nc.vector.match_replace(tk_ps[:], vals[:, 0:8], tk_ps[:], NEG)
                nc.vector.max(vals[:, 8:16], tk_ps[:])
                nc.vector.max_index(idxp[:, 8:16], vals[:, 8:16], tk_ps[:])
                # each 16-partition core group needs its own copy of the
                # wrapped index list -> duplicate before the 32x32 transpose
                nc.vector.tensor_copy(idxp[:, 16:32], idxp[:, 0:16])

                # itile4[32a+p, 32q4+w]: p in 0:16 -> idx[q=32a+w, k=p] of the
                # group's block q4 for core 2a, p in 16:32 the same indices
                # again for core 2a+1.  FOUR blocks share one index tile so
                # one ap_gather serves all four: the ucode costs ~7.3 us per
                # launch + ~13.5 ns/idx, so batching amortizes the launch.
                T = s // 4
                if s % 4 == 0:
                    itile4 = spool.tile([128, 128], u16, tag="itile")
                    itile_groups[T] = itile4
                itile4 = itile_groups[T]
                q4 = s % 4
                for a in range(4):
                    nc.vector.transpose(
                        itile4[32 * a:32 * (a + 1), 32 * q4:32 * (q4 + 1)],
                        idxp[32 * a:32 * (a + 1), :],
                    )

            def stageB(b, T):
                """Gather neighbor features for 4 blocks: chunk a of each
                block's 2048 (query, k) columns lands on partitions
                32a:32a+32, feature pairs packed as one f32 word per index
                (the d=1/f32 ucode path measured at 35 us for 2048 idxs)."""
                fp, _ = feat_tiles[b]
                nb4 = npool.tile([128, 2048], f32, tag="nb4")
                nc.gpsimd.ap_gather(
                    nb4[:],
                    fp[:].bitcast(f32),
                    itile_groups.pop(T)[:].bitcast(i16),
                    channels=128, num_elems=N, d=1, num_idxs=2048,
                )
                return nb4

            def stageC(b, blk, nb4, q4):
                """Un-scramble the gather chunks into featM: selection
                matmuls move feature 2cc+j from chunk partition 32a+cc to
                featM row cc+32j for the chunk's 512 columns (one pass for
                both h-halves); featM[64:128] gets the 16x-broadcast center
                features straight from DRAM."""
                nbv = nb4[:].bitcast(f16).rearrange("p (g j) -> p g j", j=2)
                featM = mpool.tile([128, 2048], f16, tag="featM")
                off = 512 * q4
                for a in range(4):
                    # shares the ps_b ring with the L2 tiles (same tag) so the
                    # budget stays within 8 PSUM banks with tk double-buffered
                    fm = pb.tile([128, 512], f32, tag="pb")
                    for j in range(2):
                        nc.tensor.matmul(
                            fm[0:64, :],
                            sel_sb[32 * a:32 * (a + 1), 64 * j:64 * (j + 1)],
                            nbv[32 * a:32 * (a + 1), off:off + 512, j:j + 1],
                            start=(j == 0), stop=(j == 1),
                            tile_position=(32 * a, 0),
                        )
                    if a % 2 == 0:
                        nc.scalar.activation(
                            featM[0:64, 512 * a:512 * (a + 1)], fm[0:64, :],
                            AF.Copy,
                        )
                    else:
                        # split the PSUM->SBUF drains between Act and DVE so
                        # neither engine head-of-line-blocks the psum ring
                        nc.vector.tensor_copy(
                            featM[0:64, 512 * a:512 * (a + 1)], fm[0:64, :]
                        )
                nc.sync.dma_start(
                    featM[64:128, :],
                    featB_d[b, :, blk * 2048:(blk + 1) * 2048],
                )
                return featM

            def stageD(b, blk, featM):
                """f16 MLP, single 128-deep contraction per (h-half, q-chunk):
                h1 = W1a^T nbr + (W1b-W1a)^T ctr + b1; out = gelu(W2^T gelu(h1) + b2)."""
                g2 = gpool.tile([128, 2048], f16, tag="g2")
                hs0 = hpool.tile([128, 2048], f16, tag="hs0")
                hs1 = hpool.tile([128, 2048], f16, tag="hs1")
                hs_pair = [hs0, hs1]
                for c in range(4):
                    cols = slice(512 * c, 512 * (c + 1))
                    for hh in range(2):
                        hp = ph1.tile([128, 512], f32, tag="h1ps")
                        nc.tensor.matmul(
                            hp[:],
                            w1_sb[:, hh * 128:(hh + 1) * 128],
                            featM[:, cols],
                            start=True, stop=True,
                        )
                        nc.scalar.activation(
                            hs_pair[hh][:, cols], hp[:], act_fn,
                            bias=b1_sb[:, hh:hh + 1],
                        )
                    p2 = pb.tile([128, 512], f32, tag="pb")
                    for hh in range(2):
                        nc.tensor.matmul(
                            p2[:],
                            w2_sb[:, hh * 128:(hh + 1) * 128],
                            hs_pair[hh][:, cols],
                            start=(hh == 0), stop=(hh == 1),
                        )
                    nc.scalar.activation(
                        g2[:, cols], p2[:], act_fn, bias=b2_sb[:, 0:1],
                    )
                return g2

            def stageE(b, blk, g2):
                """Mean over the k=16 neighbors, scale, store [P, r]."""
                red = spool.tile([128, 128], f32, tag="red")
                nc.vector.tensor_reduce(
                    red[:], g2[:].rearrange("p (r k) -> p r k", k=K),
                    axis=mybir.AxisListType.X, op=mybir.AluOpType.add,
                )
                outT = spool.tile([128, 128], f32, tag="outT")
                nc.vector.tensor_scalar_mul(outT[:], red[:], 1.0 / K)
                nc.sync.dma_start(out_d[b, blk], outT[:])

            S = n_b * n_blk
            B_out, C_out, D_out = {}, {}, {}
            for s in range(S + 7):
                if s < S:
                    stageA(*divmod(s, n_blk), s)
                    if s % 4 == 3:
                        T = s // 4
                        B_out[T] = stageB((4 * T) // n_blk, T)
                if 5 <= s <= S + 4:
                    sc = s - 5
                    C_out[sc] = stageC(*divmod(sc, n_blk), B_out[sc // 4], sc % 4)
                    if sc % 4 == 3:
                        B_out.pop(sc // 4)
                if 6 <= s <= S + 5:
                    D_out[s - 6] = stageD(*divmod(s - 6, n_blk), C_out.pop(s - 6))
                if s >= 7:
                    stageE(*divmod(s - 7, n_blk), D_out.pop(s - 7))

    nc.compile()
    return nc


def prep_core_inputs(points, features, W1, b1, W2, b2, core):
    """Host-side packing of one core's inputs (batches core*BL .. core*BL+BL)."""
    import ml_dtypes
    bf = ml_dtypes.bfloat16
    sl = slice(core * BL, (core + 1) * BL)
    pts = points[sl]           # [BL, N, 3]
    fts = features[sl]         # [BL, N, F]

    fT = np.ascontiguousarray(fts.transpose(0, 2, 1))        # [BL, 64, N]
    featC = fT.astype(np.float16)
    # featP[b, 32a+cc, 2n+j] = feat[b, 2cc+j, n], replicated over a=0..3
    packed = featC.reshape(BL, 32, 2, N).transpose(0, 1, 3, 2)
    featP = np.tile(packed.reshape(BL, 32, 2 * N), (1, 4, 1))
    # featB[b, f, 16n+k] = feat[b, f, n]: 16x-broadcast center features
    featB = np.repeat(featC, K, axis=2)

    r = (pts.astype(np.float64) ** 2).sum(-1).astype(np.float32)  # [BL, N]
    p_hi = pts.astype(bf).astype(np.float32)
    p_lo = (pts - p_hi).astype(bf).astype(np.float32)
    r_hi = r.astype(bf).astype(np.float32)
    r_lo = (r - r_hi).astype(bf).astype(np.float32)

    ab = np.zeros((BL, 2, 16, N), np.float32)
    # lhs rows (A) pair with rhs rows (B); Tk = 2 p_i . p_j - r_j
    ab[:, 0, 0:3] = 2.0 * p_hi.transpose(0, 2, 1)
    ab[:, 0, 3:6] = 2.0 * p_lo.transpose(0, 2, 1)
    ab[:, 0, 6:9] = 2.0 * p_hi.transpose(0, 2, 1)
    ab[:, 0, 9] = -1.0
    ab[:, 0, 10] = -1.0
    ab[:, 1, 0:3] = p_hi.transpose(0, 2, 1)
    ab[:, 1, 3:6] = p_hi.transpose(0, 2, 1)
    ab[:, 1, 6:9] = p_lo.transpose(0, 2, 1)
    ab[:, 1, 9] = r_hi
    ab[:, 1, 10] = r_lo
    ab = ab.astype(bf)

    # selection weights: move chunk partition 32a+cc, pair-lane j to featM
    # row cc+32j.  sel[32a+cc, 64j + (cc+32j)] = 1.
    sel = np.zeros((128, 128), np.float16)
    cc = np.arange(32)
    for a in range(4):
        sel[32 * a + cc, cc] = 1.0            # j=0 -> rows cc
        sel[32 * a + cc, 64 + 32 + cc] = 1.0  # j=1 -> rows 32+cc
    # w1 rows 0:64: W1a with row P holding feature 2*(P%32)+P//32 (the deint
    # order); rows 64:128: W1b - W1a in natural order.
    w1p = np.empty((128, 256), np.float32)
    w1p[0:64] = W1[0:64].reshape(32, 2, 256).transpose(1, 0, 2).reshape(64, 256)
    w1p[64:128] = W1[64:128] - W1[0:64]
    w1p = w1p.astype(np.float16)
    w2p = np.empty((128, 256), np.float32)
    w2p[:, 0:128] = W2[0:128]
    w2p[:, 128:256] = W2[128:256]
    w2p = w2p.astype(np.float16)
    b1p = np.ascontiguousarray(b1.reshape(2, 128).T)
    b2p = np.ascontiguousarray(b2.reshape(128, 1))

    eye = np.eye(128, dtype=np.float32)
    cbf = np.concatenate([eye, NEG * eye], axis=1).astype(bf)

    return {
        "featP": np.ascontiguousarray(featP),
        "featB": np.ascontiguousarray(featB),
        "ab": np.ascontiguousarray(ab),
        "sel": sel,
        "w1": w1p, "w2": w2p, "b1": b1p, "b2": b2p,
        "cbf": np.ascontiguousarray(cbf),
    }


_CACHED = {}


def kernel(points, features, W1, b1, W2, b2):
    from concourse import bass_utils

    points = np.asarray(points, np.float32)
    features = np.asarray(features, np.float32)
    W1 = np.asarray(W1, np.float32)
    b1 = np.asarray(b1, np.float32)
    W2 = np.asarray(W2, np.float32)
    b2 = np.asarray(b2, np.float32)

    if "nc" not in _CACHED:
        _CACHED["nc"] = build_program(gelu=True)
    nc = _CACHED["nc"]

    in_maps = [
        prep_core_inputs(points, features, W1, b1, W2, b2, c)
        for c in range(NCORES)
    ]
    res = bass_utils.run_bass_kernel_spmd(
        nc, in_maps, core_ids=list(range(NCORES))
    )
    outs = []
    for c in range(NCORES):
        o = res.results[c]["out"]          # [BL, NBLK, 128, 128] = [b, blk, P, r]
        outs.append(o.transpose(0, 1, 3, 2).reshape(BL, N, P))
    return np.concatenate(outs, axis=0)
